# revision 19
# baseline (speedup 1.0000x reference)
"""Trainium2 Bass kernel for AssociativeMemoryModule (causal linear attention).

Sharding: head-parallel - core c owns head c for both batches. Each core:
  1. projects full x (pre-transposed, bf16 on host) to [q.T;k.T] (128 rows)
     and v.T (64 rows); phi = min(exp(z),1) + relu(z) in f32 -> bf16,
  2. per 128-chunk: PE-transposes kT/vT to normal layout straight out of
     the phi tile (partition offset 64 for kT - no re-base DMA), scores
     via qT/kT quadrants, masked with one batched DVE mul per 512-t tile,
  3. chunked causal linear attention (C=128): kv outer products + DVE
     prefix adds reading PSUM directly; po epilogue in two waves of 4 to
     bound PSUM, out-transposes DMA'd to the A2A bounce buffer straight
     from PSUM,
  4. per-batch AllToAll redistributes head-sharded -> t-sharded (bf16);
     batch-0's collective overlaps batch-1 compute,
  5. o-projection in NORMAL orientation: lhsT = gathered head-transposed
     tiles, rhs = Wo.T tiles (512-col matmuls), bias via rank-1 matmul,
     result DMA'd PSUM -> DRAM. Output is o[b, 128 t rows, 512] per core.
Host reassembles the 8 (2, 128, 512) row-slices.
"""
import sys

import numpy as np

sys.path.insert(0, "/opt/trn_rl_repo")

H, HD, D = 8, 64, 512
B, T = 2, 1024
BT = B * T            # 2048
C = 128               # attention chunk
NCH = BT // C         # 16 chunks total
CPB = T // C          # 8 chunks per batch
NF = D // 128         # 4 feature tiles
NT = 4                # t-tiles of 512 for projections

_CACHE = {}


def _build():
    if "nc" in _CACHE:
        return _CACHE["nc"]
    import concourse.mybir as mybir
    import concourse.tile as tile
    from concourse import bacc
    from concourse.bass import ts

    import ml_dtypes

    f32 = mybir.dt.float32
    bf16 = mybir.dt.bfloat16
    AF = mybir.ActivationFunctionType

    nc = bacc.Bacc("TRN2", target_bir_lowering=False, debug=False, num_devices=8,
                   num_swdge_queues=4)

    xT = nc.declare_dram_parameter("xT", [D, BT], bf16, isOutput=False)
    wa = nc.declare_dram_parameter("wa", [D, 128], bf16, isOutput=False)
    wv = nc.declare_dram_parameter("wv", [D, HD], bf16, isOutput=False)
    won = nc.declare_dram_parameter("won", [NF, 128, D], bf16, isOutput=False)
    bqk = nc.declare_dram_parameter("bqk", [128, 1], f32, isOutput=False)
    bv = nc.declare_dram_parameter("bv", [HD, 1], f32, isOutput=False)
    bon = nc.declare_dram_parameter("bon", [1, D], bf16, isOutput=False)
    out = nc.declare_dram_parameter("out", [B, C, D], f32, isOutput=True)

    # mask4[s, jj, t] = s <= t (same causal mask for each of 4 chunks)
    mask_np = np.broadcast_to(
        np.triu(np.ones((C, C), np.float32))[:, None, :], (C, 4, C)).copy()
    iden_np = np.eye(HD, dtype=ml_dtypes.bfloat16)
    iden128_np = np.eye(C, dtype=ml_dtypes.bfloat16)
    mask_d = nc.inline_tensor(mask_np.reshape(C, 4 * C), "causal_mask4")
    iden_d = nc.inline_tensor(iden_np, "iden64")
    iden128_d = nc.inline_tensor(iden128_np, "iden128")

    with tile.TileContext(nc) as tc:
        with (
            tc.tile_pool(name="consts", bufs=1) as consts,
            tc.tile_pool(name="dram", bufs=1, space="DRAM") as dram,
        ):
            # ---- resident SBUF tensors (matmul operands in bf16) ----
            xt_sb = consts.tile([128, NF, BT], bf16)
            wa_sb = consts.tile([128, NF, 128], bf16)
            wv_sb = consts.tile([128, NF, HD], bf16)
            won_sb = consts.tile([128, NF, D], bf16)
            bqk_sb = consts.tile([128, 1], f32)
            bv_sb = consts.tile([HD, 1], f32)
            bon_sb = consts.tile([1, D], bf16)
            ones_sb = consts.tile([1, C], bf16)
            mask_sb = consts.tile([C, 4, C], f32)
            iden_sb = consts.tile([HD, HD], bf16)
            iden128_sb = consts.tile([C, C], bf16)
            qk_phi = consts.tile([128, BT], bf16)      # rows 0-63 qT, 64-127 kT
            kvT = consts.tile([128, BT], bf16)         # rows 0-63 kT, 64-127 vT
            vT_sb = consts.tile([HD, BT], bf16)
            k_nrm = consts.tile([128, NCH, HD], bf16)
            v_aug = consts.tile([128, NCH, HD + 1], bf16)
            g_sb = [consts.tile([128, NF, C], bf16, tag=f"g{b}", name=f"g{b}")
                    for b in range(B)]
            sm_all = consts.tile([C, NCH, C], bf16)
            Sf = consts.tile([HD, B, CPB - 1, HD + 1], f32)
            Sb16 = consts.tile([HD, B, CPB - 1, HD + 1], bf16)

            # merged bounce buffer: shard j = my head's out.T t-cols
            # [128j:128j+128) for BOTH batches (one A2A instead of two)
            cc_in = dram.tile([8, B, HD, C], bf16, tag="ci", name="ci")
            cc_out = dram.tile([8, B, HD, C], bf16, tag="co", name="co")

            # ---- input staging. Each dma_start blocks its queue ~600ns, so
            # the per-queue issue ORDER is the schedule: first-needed first.
            # pa(f) needs wa[f] (scalar head) + xt[f,tt0] (sync/scalar heads).
            def xt_dma(eng, tcol, f):
                eng.dma_start(xt_sb[:, f, ts(tcol, 512)],
                              xT[128 * f:128 * (f + 1), ts(tcol, 512)])

            nc.scalar.dma_start(wa_sb[:],
                                wa.ap().rearrange("(f p) c -> p f c", p=128))
            xt_dma(nc.sync, 0, 0)
            xt_dma(nc.gpsimd, 0, 1)
            xt_dma(nc.sync, 0, 2)
            xt_dma(nc.scalar, 0, 3)
            nc.scalar.dma_start(bqk_sb[:], bqk[:, :])
            nc.scalar.dma_start(bv_sb[:], bv[:, :])
            nc.gpsimd.dma_start(wv_sb[:],
                                wv.ap().rearrange("(f p) c -> p f c", p=128))
            nc.gpsimd.dma_start(iden_sb[:], iden_d[:, :])
            nc.gpsimd.dma_start(iden128_sb[:], iden128_d[:, :])
            nc.gpsimd.dma_start(mask_sb[:],
                                mask_d.ap().rearrange("p (j t) -> p j t", j=4))
            nc.vector.memset(v_aug[:, :, HD:HD + 1], 1.0)
            nc.vector.memset(ones_sb[:], 1.0)
            for tcol in range(1, NT):
                xt_dma(nc.sync, tcol, 0)
                xt_dma(nc.scalar, tcol, 1)
                xt_dma(nc.sync, tcol, 2)
                xt_dma(nc.scalar, tcol, 3)
            # o-proj weights: needed only ~60us in; issue on sync after xT
            for f in range(NF):
                nc.sync.dma_start(won_sb[:, f, :], won[f, :, :])
            nc.sync.dma_start(bon_sb[:], bon[:, :])

            with (
                tc.tile_pool(name="psA", bufs=2, space="PSUM") as psA,
                tc.tile_pool(name="psB", bufs=1, space="PSUM") as psB,
                tc.tile_pool(name="psT", bufs=2, space="PSUM") as psT,
                tc.tile_pool(name="psSc", bufs=1, space="PSUM") as psSc,
                tc.tile_pool(name="psO", bufs=2, space="PSUM") as psO,
                tc.tile_pool(name="ptmp", bufs=2) as ptmp,
                tc.tile_pool(name="attn", bufs=4) as attn,
            ):
                def proj_tile(tt):
                    sl = ts(tt, 512)
                    pa = psA.tile([128, 512], f32, tag="pa", name=f"pa{tt}")
                    pb = psB.tile([HD, 512], f32, tag="pb", name=f"pb{tt}")
                    for f in range(NF):
                        nc.tensor.matmul(pa, wa_sb[:, f, :], xt_sb[:, f, sl],
                                         start=(f == 0), stop=(f == NF - 1))
                    for f in range(NF):
                        nc.tensor.matmul(pb, wv_sb[:, f, :], xt_sb[:, f, sl],
                                         start=(f == 0), stop=(f == NF - 1))
                    nc.scalar.activation(vT_sb[:, sl], pb, AF.Identity, bias=bv_sb[:])
                    # phi = exp(min(z,0)) + relu(z) = min(exp(z),1) + relu(z):
                    # both ACT ops read PSUM directly with fused bias
                    rr = ptmp.tile([128, 512], f32, tag="rr", name=f"rr{tt}")
                    ee = ptmp.tile([128, 512], f32, tag="ee", name=f"ee{tt}")
                    mm = ptmp.tile([128, 512], f32, tag="mm", name=f"mm{tt}")
                    nc.scalar.activation(ee, pa, AF.Exp, bias=bqk_sb[:])
                    nc.scalar.activation(rr, pa, AF.Relu, bias=bqk_sb[:])
                    nc.vector.tensor_scalar_min(mm, ee, 1.0)
                    nc.vector.tensor_add(qk_phi[:, sl], mm, rr)
                    # build [kT; vT] at base partition 0 (SBUF->SBUF DMAs):
                    # matmul operands must share a base partition, and the
                    # stacked tile transposes k and v chunks in ONE PE op
                    nc.sync.dma_start(kvT[0:HD, sl], qk_phi[64:128, sl])
                    nc.scalar.dma_start(kvT[HD:128, sl], vT_sb[:, sl])
                    # transposes + scores for the 4 chunks in this t-tile
                    ptr = psT.tile([C, 4, C], bf16, tag="tr", name=f"tr{tt}")
                    psc = psSc.tile([C, 4, C], f32, tag="ps", name=f"ps{tt}")
                    for jj in range(4):
                        i = tt * 4 + jj
                        cs = ts(i, C)
                        nc.tensor.transpose(ptr[:, jj, :], kvT[:, cs],
                                            iden128_sb[:])
                        nc.tensor.matmul(psc[:, jj, :], kvT[0:HD, cs],
                                         qk_phi[0:64, cs], start=True, stop=True)
                    i0 = tt * 4
                    nc.scalar.copy(k_nrm[:, i0:i0 + 4, :], ptr[:, :, 0:HD])
                    nc.vector.tensor_copy(v_aug[:, i0:i0 + 4, 0:HD],
                                          ptr[:, :, HD:2 * HD])
                    nc.vector.tensor_mul(sm_all[:, i0:i0 + 4, :], psc, mask_sb[:])

                def kv_part(b):
                    # kv products + incremental prefix state (j = 0..CPB-2)
                    for w in range(2):
                        pkv = psO.tile([HD, 4, HD + 1], f32, tag="po",
                                       name=f"pkv{b}{w}")
                        for jw in range(4 if w == 0 else 3):
                            j = 4 * w + jw
                            i = b * CPB + j
                            nc.tensor.matmul(pkv[:, jw, :], k_nrm[:, i, :],
                                             v_aug[:, i, :], start=True, stop=True)
                        for jw in range(4 if w == 0 else 3):
                            j = 4 * w + jw
                            if j == 0:
                                nc.vector.tensor_copy(Sf[:, b, 0, :], pkv[:, 0, :])
                            else:
                                nc.vector.tensor_add(Sf[:, b, j, :],
                                                     Sf[:, b, j - 1, :],
                                                     pkv[:, jw, :])
                            nc.scalar.copy(Sb16[:, b, j, :], Sf[:, b, j, :])

                def po_part(b):
                    # two waves of 4 chunks: all po matmuls, then DVE epilogue,
                    # then out-transposes (PE never stalls on DVE), then DMA
                    # straight from PSUM to the bounce buffer
                    for w in range(2):
                        po = psO.tile([C, 4, HD + 1], f32, tag="po",
                                      name=f"po{b}{w}")
                        ptr = psT.tile([C, 2, C], bf16, tag="tr",
                                       name=f"otr{b}{w}")
                        for jw in range(4):
                            j = 4 * w + jw
                            i = b * CPB + j
                            cs = ts(i, C)
                            if j == 0:
                                nc.tensor.matmul(po[:, jw, :], sm_all[:, i, :],
                                                 v_aug[:, i, :],
                                                 start=True, stop=True)
                            else:
                                nc.tensor.matmul(po[:, jw, :], sm_all[:, i, :],
                                                 v_aug[:, i, :],
                                                 start=True, stop=False)
                                nc.tensor.matmul(po[:, jw, :], qk_phi[0:64, cs],
                                                 Sb16[:, b, j - 1, :],
                                                 start=False, stop=True)
                        on2 = [attn.tile([C, 2 * HD], bf16, tag=f"on{p}",
                                         name=f"on{b}{w}{p}") for p in range(2)]
                        for jw in range(4):
                            j = 4 * w + jw
                            i = b * CPB + j
                            # denom > 0 always (phi > 0); reference's 1e-6
                            # clamp can never bind at these magnitudes
                            dr = attn.tile([C, 1], f32, tag="dr", name=f"dr{i}")
                            nc.vector.reciprocal(dr, po[:, jw, HD:HD + 1])
                            nc.vector.tensor_scalar_mul(
                                on2[jw // 2][:, HD * (jw % 2):HD * (jw % 2 + 1)],
                                po[:, jw, 0:HD], dr)
                            if jw % 2 == 1:
                                # pair done: one transpose covers 2 chunks
                                nc.tensor.transpose(ptr[:, jw // 2, :],
                                                    on2[jw // 2][:],
                                                    iden128_sb[:])
                        ot = attn.tile([C, 2, C], bf16, tag="ot", name=f"ot{b}{w}")
                        nc.scalar.copy(ot[:], ptr[:])
                        for p in range(2):
                            nc.sync.dma_start(
                                cc_in.rearrange("(q lo) b m t -> lo m b q t",
                                                lo=2)[:, :, b, 2 * w + p, :],
                                ot[:, p, :])

                def trigger():
                    nc.gpsimd.collective_compute(
                        "AllToAll",
                        mybir.AluOpType.bypass,
                        replica_groups=[list(range(8))],
                        ins=[cc_in.opt()],
                        outs=[cc_out.opt()],
                    )

                def oproj_batch(b):
                    # gathered heads, partition-packed in pairs: 2 DMAs/batch.
                    # o-proj in normal orientation: lhsT = onT tiles, rhs =
                    # Wo.T tiles, bias via rank-1 ones x bon matmul; result
                    # [128 t, 512] via SBUF bounce.
                    co = cc_out.rearrange("(ki two) b m t -> b two m ki t", two=2)
                    eng = nc.scalar if b == 0 else nc.sync
                    eng.dma_start(g_sb[b][0:64, :, :], co[b][0])
                    eng.dma_start(g_sb[b][64:128, :, :], co[b][1])
                    pf = psA.tile([128, 512], f32, tag="pa", name=f"pf{b}")
                    for ki in range(NF):
                        nc.tensor.matmul(pf, g_sb[b][:, ki, :], won_sb[:, ki, :],
                                         start=(ki == 0), stop=False)
                    nc.tensor.matmul(pf, ones_sb[:], bon_sb[:],
                                     start=False, stop=True)
                    osl = attn.tile([128, 512], f32, tag="osl", name=f"osl{b}")
                    for h in range(2):
                        nc.scalar.copy(osl[:, ts(h, 256)], pf[:, ts(h, 256)])
                        nc.sync.dma_start(out[b, :, ts(h, 256)],
                                          osl[:, ts(h, 256)])

                proj_tile(0)
                proj_tile(1)
                kv_part(0)
                po_part(0)
                proj_tile(2)
                proj_tile(3)
                kv_part(1)
                po_part(1)
                trigger()
                oproj_batch(0)
                oproj_batch(1)

    nc.compile()
    _CACHE["nc"] = nc
    return nc


def _in_maps(x, Wq, bq, Wk, bk, Wv, bv, Wo, bo):
    import ml_dtypes
    bf = ml_dtypes.bfloat16
    x2 = np.ascontiguousarray(x.reshape(BT, D).T).astype(bf)
    # won[ki, 64*two + m, d] = Wo[d, 64*(2ki+two)+m] - matches g_sb packing
    WoT = np.ascontiguousarray(Wo.T)                  # [(h m), d]
    won = WoT.reshape(NF, 2, HD, D).transpose(0, 1, 2, 3)  # [ki, two, m, d]
    won = np.ascontiguousarray(won.reshape(NF, 128, D)).astype(bf)
    bon = np.ascontiguousarray(bo.reshape(1, D)).astype(bf)
    maps = []
    for c in range(8):
        sl = slice(HD * c, HD * (c + 1))
        maps.append(dict(
            xT=x2,
            wa=np.ascontiguousarray(np.concatenate([Wq[sl], Wk[sl]], 0).T).astype(bf),
            wv=np.ascontiguousarray(Wv[sl].T).astype(bf),
            won=won,
            bqk=np.ascontiguousarray(np.concatenate([bq[sl], bk[sl]]).reshape(128, 1)).astype(np.float32),
            bv=np.ascontiguousarray(bv[sl].reshape(HD, 1)).astype(np.float32),
            bon=bon,
        ))
    return maps


def kernel(x, Wq, bq, Wk, bk, Wv, bv, Wo, bo):
    from concourse import bass_utils

    nc = _build()
    maps = _in_maps(np.asarray(x), np.asarray(Wq), np.asarray(bq),
                    np.asarray(Wk), np.asarray(bk), np.asarray(Wv),
                    np.asarray(bv), np.asarray(Wo), np.asarray(bo))
    res = bass_utils.run_bass_kernel_spmd(nc, maps, core_ids=list(range(8)))
    o = np.zeros((B, T, D), np.float32)
    for c in range(8):
        s = res.results[c]["out"]                     # (2, 128, 512)
        o[0, C * c:C * (c + 1), :] = s[0]
        o[1, C * c:C * (c + 1), :] = s[1]
    return np.ascontiguousarray(o).astype(np.float32)


# revision 20
# speedup vs baseline: 1.0828x; 1.0828x over previous
"""Trainium2 Bass kernel for AssociativeMemoryModule (causal linear attention).

Sharding: head-parallel - core c owns head c for both batches. Each core:
  1. projects full x (pre-transposed, bf16 on host) to [q.T;k.T] (128 rows)
     and v.T (64 rows); phi = min(exp(z),1) + relu(z) in f32 -> bf16,
  2. per 128-chunk: PE-transposes kT/vT to normal layout straight out of
     the phi tile (partition offset 64 for kT - no re-base DMA), scores
     via qT/kT quadrants, masked with one batched DVE mul per 512-t tile,
  3. chunked causal linear attention (C=128): kv outer products + DVE
     prefix adds reading PSUM directly; po epilogue in two waves of 4 to
     bound PSUM, out-transposes DMA'd to the A2A bounce buffer straight
     from PSUM,
  4. per-batch AllToAll redistributes head-sharded -> t-sharded (bf16);
     batch-0's collective overlaps batch-1 compute,
  5. o-projection in NORMAL orientation: lhsT = gathered head-transposed
     tiles, rhs = Wo.T tiles (512-col matmuls), bias via rank-1 matmul,
     result DMA'd PSUM -> DRAM. Output is o[b, 128 t rows, 512] per core.
Host reassembles the 8 (2, 128, 512) row-slices.
"""
import sys

import numpy as np

sys.path.insert(0, "/opt/trn_rl_repo")

H, HD, D = 8, 64, 512
B, T = 2, 1024
BT = B * T            # 2048
C = 128               # attention chunk
NCH = BT // C         # 16 chunks total
CPB = T // C          # 8 chunks per batch
NF = D // 128         # 4 feature tiles
NT = 4                # t-tiles of 512 for projections

_CACHE = {}


def _build():
    if "nc" in _CACHE:
        return _CACHE["nc"]
    import concourse.mybir as mybir
    import concourse.tile as tile
    from concourse import bacc
    from concourse.bass import ts

    import ml_dtypes

    f32 = mybir.dt.float32
    bf16 = mybir.dt.bfloat16
    AF = mybir.ActivationFunctionType

    nc = bacc.Bacc("TRN2", target_bir_lowering=False, debug=False, num_devices=8,
                   num_swdge_queues=4)

    xT = nc.declare_dram_parameter("xT", [D, BT], bf16, isOutput=False)
    wa = nc.declare_dram_parameter("wa", [D, 128], bf16, isOutput=False)
    wv = nc.declare_dram_parameter("wv", [D, HD], bf16, isOutput=False)
    won = nc.declare_dram_parameter("won", [NF, 128, D], bf16, isOutput=False)
    bqk = nc.declare_dram_parameter("bqk", [128, 1], f32, isOutput=False)
    bv = nc.declare_dram_parameter("bv", [HD, 1], f32, isOutput=False)
    bon = nc.declare_dram_parameter("bon", [1, D], bf16, isOutput=False)
    out = nc.declare_dram_parameter("out", [B, C, D], f32, isOutput=True)

    # mask4[s, jj, t] = s <= t (same causal mask for each of 4 chunks)
    mask_np = np.broadcast_to(
        np.triu(np.ones((C, C), np.float32))[:, None, :], (C, 4, C)).copy()
    iden_np = np.eye(HD, dtype=ml_dtypes.bfloat16)
    iden128_np = np.eye(C, dtype=ml_dtypes.bfloat16)
    mask_d = nc.inline_tensor(mask_np.reshape(C, 4 * C), "causal_mask4")
    iden_d = nc.inline_tensor(iden_np, "iden64")
    iden128_d = nc.inline_tensor(iden128_np, "iden128")

    with tile.TileContext(nc) as tc:
        with (
            tc.tile_pool(name="consts", bufs=1) as consts,
            tc.tile_pool(name="dram", bufs=1, space="DRAM") as dram,
        ):
            # ---- resident SBUF tensors (matmul operands in bf16) ----
            xt_sb = consts.tile([128, NF, BT], bf16)
            wa_sb = consts.tile([128, NF, 128], bf16)
            wv_sb = consts.tile([128, NF, HD], bf16)
            won_sb = consts.tile([128, NF, D], bf16)
            bqk_sb = consts.tile([128, 1], f32)
            bv_sb = consts.tile([HD, 1], f32)
            bon_sb = consts.tile([1, D], bf16)
            ones_sb = consts.tile([1, C], bf16)
            mask_sb = consts.tile([C, 4, C], f32)
            iden_sb = consts.tile([HD, HD], bf16)
            iden128_sb = consts.tile([C, C], bf16)
            qk_phi = consts.tile([128, BT], bf16)      # rows 0-63 qT, 64-127 kT
            kvT = consts.tile([128, BT], bf16)         # rows 0-63 kT, 64-127 vT
            vT_sb = consts.tile([HD, BT], bf16)
            k_nrm = consts.tile([128, NCH, HD], bf16)
            v_aug = consts.tile([128, NCH, HD + 1], bf16)
            g_sb = [consts.tile([128, NF, C], bf16, tag=f"g{b}", name=f"g{b}")
                    for b in range(B)]
            sm_all = consts.tile([C, NCH, C], bf16)
            Sf = consts.tile([HD, B, CPB - 1, HD + 1], f32)
            Sb16 = consts.tile([HD, B, CPB - 1, HD + 1], bf16)

            # merged bounce buffer: shard j = my head's out.T t-cols
            # [128j:128j+128) for BOTH batches (one A2A instead of two)
            cc_in = dram.tile([8, B, HD, C], bf16, tag="ci", name="ci")
            cc_out = dram.tile([8, B, HD, C], bf16, tag="co", name="co")

            # ---- input staging. Each dma_start blocks its queue ~600ns, so
            # the per-queue issue ORDER is the schedule: first-needed first.
            # pa(f) needs wa[f] (scalar head) + xt[f,tt0] (sync/scalar heads).
            def xt_dma(eng, tcol, f):
                eng.dma_start(xt_sb[:, f, ts(tcol, 512)],
                              xT[128 * f:128 * (f + 1), ts(tcol, 512)])

            nc.scalar.dma_start(wa_sb[:, 0, :], wa[0:128, :])
            nc.scalar.dma_start(wa_sb[:, 1, :], wa[128:256, :])
            xt_dma(nc.sync, 0, 0)
            xt_dma(nc.gpsimd, 0, 1)
            nc.scalar.dma_start(wa_sb[:, 2, :], wa[256:384, :])
            nc.scalar.dma_start(wa_sb[:, 3, :], wa[384:512, :])
            xt_dma(nc.sync, 0, 2)
            xt_dma(nc.scalar, 0, 3)
            nc.scalar.dma_start(bqk_sb[:], bqk[:, :])
            nc.scalar.dma_start(bv_sb[:], bv[:, :])
            for f in range(NF):
                nc.gpsimd.dma_start(wv_sb[:, f, :], wv[128 * f:128 * (f + 1), :])
            nc.gpsimd.dma_start(iden_sb[:], iden_d[:, :])
            nc.gpsimd.dma_start(iden128_sb[:], iden128_d[:, :])
            nc.gpsimd.dma_start(mask_sb[:],
                                mask_d.ap().rearrange("p (j t) -> p j t", j=4))
            nc.vector.memset(v_aug[:, :, HD:HD + 1], 1.0)
            nc.vector.memset(ones_sb[:], 1.0)
            for tcol in range(1, NT):
                xt_dma(nc.sync, tcol, 0)
                xt_dma(nc.scalar, tcol, 1)
                xt_dma(nc.sync, tcol, 2)
                xt_dma(nc.scalar, tcol, 3)
            # o-proj weights: needed only ~60us in; issue on sync after xT
            for f in range(NF):
                nc.sync.dma_start(won_sb[:, f, :], won[f, :, :])
            nc.sync.dma_start(bon_sb[:], bon[:, :])

            with (
                tc.tile_pool(name="psA", bufs=2, space="PSUM") as psA,
                tc.tile_pool(name="psB", bufs=1, space="PSUM") as psB,
                tc.tile_pool(name="psT", bufs=2, space="PSUM") as psT,
                tc.tile_pool(name="psSc", bufs=1, space="PSUM") as psSc,
                tc.tile_pool(name="psO", bufs=2, space="PSUM") as psO,
                tc.tile_pool(name="ptmp", bufs=2) as ptmp,
                tc.tile_pool(name="attn", bufs=4) as attn,
            ):
                def proj_tile(tt):
                    sl = ts(tt, 512)
                    pa = psA.tile([128, 512], f32, tag="pa", name=f"pa{tt}")
                    pb = psB.tile([HD, 512], f32, tag="pb", name=f"pb{tt}")
                    for f in range(NF):
                        nc.tensor.matmul(pa, wa_sb[:, f, :], xt_sb[:, f, sl],
                                         start=(f == 0), stop=(f == NF - 1))
                    for f in range(NF):
                        nc.tensor.matmul(pb, wv_sb[:, f, :], xt_sb[:, f, sl],
                                         start=(f == 0), stop=(f == NF - 1))
                    nc.scalar.activation(vT_sb[:, sl], pb, AF.Identity, bias=bv_sb[:])
                    # phi = exp(min(z,0)) + relu(z) = min(exp(z),1) + relu(z):
                    # both ACT ops read PSUM directly with fused bias
                    rr = ptmp.tile([128, 512], f32, tag="rr", name=f"rr{tt}")
                    ee = ptmp.tile([128, 512], f32, tag="ee", name=f"ee{tt}")
                    mm = ptmp.tile([128, 512], f32, tag="mm", name=f"mm{tt}")
                    nc.scalar.activation(ee, pa, AF.Exp, bias=bqk_sb[:])
                    nc.scalar.activation(rr, pa, AF.Relu, bias=bqk_sb[:])
                    nc.vector.tensor_scalar_min(mm, ee, 1.0)
                    nc.vector.tensor_add(qk_phi[:, sl], mm, rr)
                    # build [kT; vT] at base partition 0 (SBUF->SBUF DMAs):
                    # matmul operands must share a base partition, and the
                    # stacked tile transposes k and v chunks in ONE PE op
                    nc.sync.dma_start(kvT[0:HD, sl], qk_phi[64:128, sl])
                    nc.scalar.dma_start(kvT[HD:128, sl], vT_sb[:, sl])
                    # transposes + scores for the 4 chunks in this t-tile
                    ptr = psT.tile([C, 4, C], bf16, tag="tr", name=f"tr{tt}")
                    psc = psSc.tile([C, 4, C], f32, tag="ps", name=f"ps{tt}")
                    for jj in range(4):
                        i = tt * 4 + jj
                        cs = ts(i, C)
                        nc.tensor.transpose(ptr[:, jj, :], kvT[:, cs],
                                            iden128_sb[:])
                        nc.tensor.matmul(psc[:, jj, :], kvT[0:HD, cs],
                                         qk_phi[0:64, cs], start=True, stop=True)
                    i0 = tt * 4
                    nc.scalar.copy(k_nrm[:, i0:i0 + 4, :], ptr[:, :, 0:HD])
                    nc.vector.tensor_copy(v_aug[:, i0:i0 + 4, 0:HD],
                                          ptr[:, :, HD:2 * HD])
                    nc.vector.tensor_mul(sm_all[:, i0:i0 + 4, :], psc, mask_sb[:])

                def kv_part(b):
                    # kv products + incremental prefix state (j = 0..CPB-2)
                    for w in range(2):
                        pkv = psO.tile([HD, 4, HD + 1], f32, tag="po",
                                       name=f"pkv{b}{w}")
                        for jw in range(4 if w == 0 else 3):
                            j = 4 * w + jw
                            i = b * CPB + j
                            nc.tensor.matmul(pkv[:, jw, :], k_nrm[:, i, :],
                                             v_aug[:, i, :], start=True, stop=True)
                        for jw in range(4 if w == 0 else 3):
                            j = 4 * w + jw
                            if j == 0:
                                nc.vector.tensor_copy(Sf[:, b, 0, :], pkv[:, 0, :])
                            else:
                                nc.vector.tensor_add(Sf[:, b, j, :],
                                                     Sf[:, b, j - 1, :],
                                                     pkv[:, jw, :])
                            nc.scalar.copy(Sb16[:, b, j, :], Sf[:, b, j, :])

                def po_part(b):
                    # two waves of 4 chunks: all po matmuls, then DVE epilogue,
                    # then out-transposes (PE never stalls on DVE), then DMA
                    # straight from PSUM to the bounce buffer
                    for w in range(2):
                        po = psO.tile([C, 4, HD + 1], f32, tag="po",
                                      name=f"po{b}{w}")
                        ptr = psT.tile([C, 2, C], bf16, tag="tr",
                                       name=f"otr{b}{w}")
                        for jw in range(4):
                            j = 4 * w + jw
                            i = b * CPB + j
                            cs = ts(i, C)
                            if j == 0:
                                nc.tensor.matmul(po[:, jw, :], sm_all[:, i, :],
                                                 v_aug[:, i, :],
                                                 start=True, stop=True)
                            else:
                                nc.tensor.matmul(po[:, jw, :], sm_all[:, i, :],
                                                 v_aug[:, i, :],
                                                 start=True, stop=False)
                                nc.tensor.matmul(po[:, jw, :], qk_phi[0:64, cs],
                                                 Sb16[:, b, j - 1, :],
                                                 start=False, stop=True)
                        on2 = [attn.tile([C, 2 * HD], bf16, tag=f"on{p}",
                                         name=f"on{b}{w}{p}") for p in range(2)]
                        for jw in range(4):
                            j = 4 * w + jw
                            i = b * CPB + j
                            # denom > 0 always (phi > 0); reference's 1e-6
                            # clamp can never bind at these magnitudes
                            dr = attn.tile([C, 1], f32, tag="dr", name=f"dr{i}")
                            nc.vector.reciprocal(dr, po[:, jw, HD:HD + 1])
                            nc.vector.tensor_scalar_mul(
                                on2[jw // 2][:, HD * (jw % 2):HD * (jw % 2 + 1)],
                                po[:, jw, 0:HD], dr)
                            if jw % 2 == 1:
                                # pair done: one transpose covers 2 chunks
                                nc.tensor.transpose(ptr[:, jw // 2, :],
                                                    on2[jw // 2][:],
                                                    iden128_sb[:])
                        ot = attn.tile([C, 2, C], bf16, tag="ot", name=f"ot{b}{w}")
                        nc.scalar.copy(ot[:], ptr[:])
                        for p in range(2):
                            nc.sync.dma_start(
                                cc_in.rearrange("(q lo) b m t -> lo m b q t",
                                                lo=2)[:, :, b, 2 * w + p, :],
                                ot[:, p, :])

                def trigger():
                    nc.gpsimd.collective_compute(
                        "AllToAll",
                        mybir.AluOpType.bypass,
                        replica_groups=[list(range(8))],
                        ins=[cc_in.opt()],
                        outs=[cc_out.opt()],
                    )

                def oproj_batch(b):
                    # gathered heads, partition-packed in pairs: 2 DMAs/batch.
                    # o-proj in normal orientation: lhsT = onT tiles, rhs =
                    # Wo.T tiles, bias via rank-1 ones x bon matmul; result
                    # [128 t, 512] via SBUF bounce.
                    co = cc_out.rearrange("(ki two) b m t -> b two m ki t", two=2)
                    eng = nc.scalar if b == 0 else nc.sync
                    eng.dma_start(g_sb[b][0:64, :, :], co[b][0])
                    eng.dma_start(g_sb[b][64:128, :, :], co[b][1])
                    pf = psA.tile([128, 512], f32, tag="pa", name=f"pf{b}")
                    for ki in range(NF):
                        nc.tensor.matmul(pf, g_sb[b][:, ki, :], won_sb[:, ki, :],
                                         start=(ki == 0), stop=False)
                    nc.tensor.matmul(pf, ones_sb[:], bon_sb[:],
                                     start=False, stop=True)
                    osl = attn.tile([128, 512], f32, tag="osl", name=f"osl{b}")
                    for h in range(2):
                        nc.scalar.copy(osl[:, ts(h, 256)], pf[:, ts(h, 256)])
                        nc.sync.dma_start(out[b, :, ts(h, 256)],
                                          osl[:, ts(h, 256)])

                proj_tile(0)
                proj_tile(1)
                kv_part(0)
                po_part(0)
                proj_tile(2)
                proj_tile(3)
                kv_part(1)
                po_part(1)
                trigger()
                oproj_batch(0)
                oproj_batch(1)

    nc.compile()
    _CACHE["nc"] = nc
    return nc


def _in_maps(x, Wq, bq, Wk, bk, Wv, bv, Wo, bo):
    import ml_dtypes
    bf = ml_dtypes.bfloat16
    x2 = np.ascontiguousarray(x.reshape(BT, D).T).astype(bf)
    # won[ki, 64*two + m, d] = Wo[d, 64*(2ki+two)+m] - matches g_sb packing
    WoT = np.ascontiguousarray(Wo.T)                  # [(h m), d]
    won = WoT.reshape(NF, 2, HD, D).transpose(0, 1, 2, 3)  # [ki, two, m, d]
    won = np.ascontiguousarray(won.reshape(NF, 128, D)).astype(bf)
    bon = np.ascontiguousarray(bo.reshape(1, D)).astype(bf)
    maps = []
    for c in range(8):
        sl = slice(HD * c, HD * (c + 1))
        maps.append(dict(
            xT=x2,
            wa=np.ascontiguousarray(np.concatenate([Wq[sl], Wk[sl]], 0).T).astype(bf),
            wv=np.ascontiguousarray(Wv[sl].T).astype(bf),
            won=won,
            bqk=np.ascontiguousarray(np.concatenate([bq[sl], bk[sl]]).reshape(128, 1)).astype(np.float32),
            bv=np.ascontiguousarray(bv[sl].reshape(HD, 1)).astype(np.float32),
            bon=bon,
        ))
    return maps


def kernel(x, Wq, bq, Wk, bk, Wv, bv, Wo, bo):
    from concourse import bass_utils

    nc = _build()
    maps = _in_maps(np.asarray(x), np.asarray(Wq), np.asarray(bq),
                    np.asarray(Wk), np.asarray(bk), np.asarray(Wv),
                    np.asarray(bv), np.asarray(Wo), np.asarray(bo))
    res = bass_utils.run_bass_kernel_spmd(nc, maps, core_ids=list(range(8)))
    o = np.zeros((B, T, D), np.float32)
    for c in range(8):
        s = res.results[c]["out"]                     # (2, 128, 512)
        o[0, C * c:C * (c + 1), :] = s[0]
        o[1, C * c:C * (c + 1), :] = s[1]
    return np.ascontiguousarray(o).astype(np.float32)
